# revision 8
# baseline (speedup 1.0000x reference)
"""Multi-head attention + output projection on 8 Trainium2 NeuronCores.

Problem (hardcoded): B=2, N=S=2048, DIM=1024, 8 heads, head_dim=128, fp32.
  out = softmax(Q Kt / sqrt(128)) V  -> reshape -> @ proj_w.T + proj_b

Sharding: data parallel on batch (2) x tensor parallel on heads (4 groups of
2 heads).  Each core computes attention for its 2 heads plus the partial
output projection restricted to its heads' columns; the host sums the 4
partial projections per batch and adds the bias.

Per-core kernel layout (all matmuls in float32r):
  S^T = K @ Q^T computed per 128-row s-chunk with s on partitions, so the
  softmax denominator never needs an on-chip transpose of P:
    exp on ScalarE (PSUM->SBUF, fused with the scale via host pre-scaling of Q),
    P^T @ nothing: out^T = V^T @ expS^T accumulated in PSUM,
    row sums via chunk-accumulate on VectorE + a ones-vector matmul,
    normalization deferred to out^T via a reciprocal broadcast matmul.
  Projection: Y = X @ W^T accumulated over the core's 2 head-chunks.
"""

import sys

sys.path.insert(0, "/opt/trn_rl_repo")

import numpy as np

import concourse.bass as bass  # noqa: F401  (engine namespaces live on nc)
import concourse.mybir as mybir
import concourse.tile as tile
from concourse import bacc
from concourse.bass_utils import run_bass_kernel_spmd

B = 2
N = 2048
S = 2048
DIM = 1024
NUM_HEADS = 8
HD = 128
N_CORES = 8
HEADS_PER_CORE = 2  # 4-way head parallel x 2-way batch parallel
HG = DIM // (NUM_HEADS // HEADS_PER_CORE)  # 256 dims per core
P = 128
SC = S // P  # 16 s-chunks
NQ = N // 512  # 4 query-column blocks of 512
F32 = mybir.dt.float32
F32R = mybir.dt.float32r

_nc_cache = {}


def _round_f32r(a):
    """Round fp32 to the fp32r grid (11 mantissa bits, low 12 bits zero)."""
    u = np.ascontiguousarray(a, dtype=np.float32).view(np.uint32)
    u = (u + 0x800) & np.uint32(0xFFFFF000)
    return u.view(np.float32)


def _build():
    nc = bacc.Bacc(None, target_bir_lowering=False, debug=False, num_devices=1)

    qt = nc.dram_tensor("qt", [HG, N], F32R, kind="ExternalInput").ap()
    kt = nc.dram_tensor("kt", [HG, S], F32R, kind="ExternalInput").ap()
    v = nc.dram_tensor("v", [S, HG], F32R, kind="ExternalInput").ap()
    wt = nc.dram_tensor("wt", [HG, DIM], F32R, kind="ExternalInput").ap()
    out = nc.dram_tensor("out", [N, DIM], F32, kind="ExternalOutput").ap()

    EXPF = mybir.ActivationFunctionType.Exp

    with tile.TileContext(nc) as tc:
        with (
            tc.tile_pool(name="persist", bufs=1) as persist,
            tc.tile_pool(name="e_pool", bufs=3) as e_pool,
            tc.tile_pool(name="a_pool", bufs=2) as a_pool,
            tc.tile_pool(name="small", bufs=2) as small,
            tc.tile_pool(name="y_pool", bufs=3) as y_pool,
            tc.tile_pool(name="s_ps_pool", bufs=2, space="PSUM") as s_ps_pool,
            tc.tile_pool(name="acc_ps_pool", bufs=2, space="PSUM") as acc_ps_pool,
            tc.tile_pool(name="r_ps_pool", bufs=1, space="PSUM") as r_ps_pool,
            tc.tile_pool(name="rb_ps_pool", bufs=1, space="PSUM") as rb_ps_pool,
        ):
            # Resident inputs: one big DMA each.
            qt_sb = persist.tile([P, HEADS_PER_CORE, N], F32R)
            nc.sync.dma_start(out=qt_sb, in_=qt.rearrange("(h p) n -> p h n", p=P))
            kt_sb = persist.tile([P, HEADS_PER_CORE, S], F32R)
            nc.sync.dma_start(out=kt_sb, in_=kt.rearrange("(h p) s -> p h s", p=P))
            v_sb = persist.tile([P, HEADS_PER_CORE, SC, HD], F32R)
            nc.sync.dma_start(
                out=v_sb, in_=v.rearrange("(c p) (h d) -> p h c d", p=P, h=HEADS_PER_CORE)
            )
            wt_sb = persist.tile([P, HEADS_PER_CORE, DIM], F32R)
            nc.sync.dma_start(out=wt_sb, in_=wt.rearrange("(h p) o -> p h o", p=P))

            ones_dram = nc.inline_tensor(np.ones((1, P), np.float32), name="ones_const")
            ones_col = persist.tile([P, 1], F32R)
            nc.sync.dma_start(
                out=ones_col, in_=ones_dram.ap().bitcast(F32R).rearrange("o p -> p o")
            )
            ones_row = persist.tile([1, P], F32R)
            nc.sync.dma_start(out=ones_row, in_=ones_dram.ap().bitcast(F32R))

            # X^T: normalized attention outputs, head-dim on partitions.
            xt_sb = persist.tile([P, HEADS_PER_CORE, N], F32R)

            for h in range(HEADS_PER_CORE):
                for nq in range(NQ):
                    q_blk = qt_sb[:, h, nq * 512 : (nq + 1) * 512]
                    o_ps = acc_ps_pool.tile([P, 512], F32, tag="acc")
                    a2 = a_pool.tile([P, 2, 512], F32, tag="a2")
                    for g in range(SC // 2):
                        s_ps = s_ps_pool.tile([P, 2, 512], F32, tag="s")
                        for j in range(2):
                            si = 2 * g + j
                            nc.tensor.matmul(
                                s_ps[:, j, :],
                                kt_sb[:, h, si * P : (si + 1) * P],
                                q_blk,
                                start=True,
                                stop=True,
                            )
                        e_t = e_pool.tile([P, 2, 512], F32R, tag="e")
                        nc.scalar.activation(out=e_t, in_=s_ps, func=EXPF)
                        for j in range(2):
                            si = 2 * g + j
                            nc.tensor.matmul(
                                o_ps,
                                v_sb[:, h, si, :],
                                e_t[:, j, :],
                                start=(si == 0),
                                stop=(si == SC - 1),
                            )
                        if g == 0:
                            nc.vector.tensor_copy(a2, e_t.bitcast(F32))
                        else:
                            nc.vector.tensor_add(a2, a2, e_t.bitcast(F32))
                    a1 = a_pool.tile([P, 512], F32R, tag="a1")
                    with nc.allow_low_precision(
                        reason="fp32r rowsum feed; 11-bit mantissa is the matmul input grid anyway"
                    ):
                        nc.vector.tensor_add(a1, a2[:, 0, :], a2[:, 1, :])
                    r_ps = r_ps_pool.tile([1, 512], F32, tag="r")
                    nc.tensor.matmul(r_ps, ones_col, a1, start=True, stop=True)
                    recip = small.tile([1, 512], F32R, tag="recip")
                    with nc.allow_low_precision(
                        reason="softmax denominators are O(2048); fp32r keeps ~3e-4 rel"
                    ):
                        nc.vector.reciprocal(recip, r_ps)
                    rb_ps = rb_ps_pool.tile([P, 512], F32, tag="rb")
                    nc.tensor.matmul(
                        rb_ps, ones_row, recip, start=True, stop=True
                    )
                    rb_sb = small.tile([P, 512], F32, tag="rb_sb")
                    nc.vector.tensor_copy(rb_sb, rb_ps)
                    nc.vector.tensor_mul(
                        xt_sb[:, h, nq * 512 : (nq + 1) * 512], o_ps, rb_sb
                    )

            # Partial projection: Y = X @ W^T over this core's 256 dims.
            for nt in range(N // P):
                y_sb = y_pool.tile([P, DIM], F32, tag="y")
                for ot in range(2):
                    y_ps = acc_ps_pool.tile([P, 512], F32, tag="acc")
                    for h in range(HEADS_PER_CORE):
                        nc.tensor.matmul(
                            y_ps,
                            xt_sb[:, h, nt * P : (nt + 1) * P],
                            wt_sb[:, h, ot * 512 : (ot + 1) * 512],
                            start=(h == 0),
                            stop=(h == HEADS_PER_CORE - 1),
                        )
                    nc.vector.tensor_copy(y_sb[:, ot * 512 : (ot + 1) * 512], y_ps)
                nc.sync.dma_start(out=out[nt * P : (nt + 1) * P, :], in_=y_sb)

    nc.compile()
    return nc


def kernel(query, key, value, proj_w, proj_b):
    if "nc" not in _nc_cache:
        _nc_cache["nc"] = _build()
    nc = _nc_cache["nc"]

    scale = float(HD) ** -0.5
    wt_full = np.ascontiguousarray(proj_w.T.astype(np.float32))  # [in, out]
    in_maps = []
    for core in range(N_CORES):
        b, hg = divmod(core, N_CORES // B)
        sl = slice(hg * HG, (hg + 1) * HG)
        in_maps.append(
            {
                "qt": _round_f32r(query[b].T[sl] * scale),
                "kt": _round_f32r(key[b].T[sl]),
                "v": _round_f32r(value[b][:, sl]),
                "wt": _round_f32r(wt_full[sl]),
            }
        )

    res = run_bass_kernel_spmd(nc, in_maps, list(range(N_CORES)))

    out = np.zeros((B, N, DIM), dtype=np.float32)
    for core in range(N_CORES):
        b = core // (N_CORES // B)
        out[b] += res.results[core]["out"]
    out += proj_b.astype(np.float32)
    return out


# revision 9
# speedup vs baseline: 1.2944x; 1.2944x over previous
"""Multi-head attention + output projection on 8 Trainium2 NeuronCores.

Problem (hardcoded): B=2, N=S=2048, DIM=1024, 8 heads, head_dim=128, fp32.
  out = softmax(Q K^T / sqrt(128)) V  -> reshape -> @ proj_w.T + proj_b

Sharding: data parallel on batch (2) x tensor parallel on heads (4 groups of
2 heads).  Each core computes attention for its 2 heads plus the partial
output projection restricted to its heads' columns; the host sums the 4
partial projections per batch and adds the bias.

Per-core kernel (matmul operands in fp16, accumulation in fp32 PSUM):
  S^T = K @ Q^T per 128-row s-chunk with s on partitions, so softmax needs
  no on-chip transpose of P: exp on ScalarE (PSUM->SBUF, scale pre-applied
  to Q on host), out^T = V^T @ expS^T accumulated in PSUM.  Row sums:
  chunk-accumulate expS^T on VectorE, then one all-ones [128x128] matmul
  yields the partition-dim colsum broadcast to all 128 partitions;
  reciprocal + multiply normalizes out^T.  Projection: Y = X @ W^T
  accumulated over the core's 2 head-chunks.
"""

import sys

sys.path.insert(0, "/opt/trn_rl_repo")

import numpy as np

import concourse.bass as bass  # noqa: F401  (engine namespaces live on nc)
import concourse.mybir as mybir
import concourse.tile as tile
from concourse import bacc
from concourse.bass_utils import run_bass_kernel_spmd

B = 2
N = 2048
S = 2048
DIM = 1024
NUM_HEADS = 8
HD = 128
N_CORES = 8
HEADS_PER_CORE = 2  # 4-way head parallel x 2-way batch parallel
HG = DIM // (NUM_HEADS // HEADS_PER_CORE)  # 256 dims per core
P = 128
SC = S // P  # 16 s-chunks
NB = 512  # query-column block
NQ = N // NB
F32 = mybir.dt.float32
F16 = mybir.dt.float16

_nc_cache = {}


def _build():
    nc = bacc.Bacc(None, target_bir_lowering=False, debug=False, num_devices=1)

    qt = nc.dram_tensor("qt", [HG, N], F16, kind="ExternalInput").ap()
    kt = nc.dram_tensor("kt", [HG, S], F16, kind="ExternalInput").ap()
    v = nc.dram_tensor("v", [S, HG], F16, kind="ExternalInput").ap()
    wt = nc.dram_tensor("wt", [HG, DIM], F16, kind="ExternalInput").ap()
    out = nc.dram_tensor("out", [N, DIM], F32, kind="ExternalOutput").ap()

    EXPF = mybir.ActivationFunctionType.Exp

    with tile.TileContext(nc) as tc:
        with (
            tc.tile_pool(name="persist", bufs=1) as persist,
            tc.tile_pool(name="e_pool", bufs=3) as e_pool,
            tc.tile_pool(name="a_pool", bufs=2) as a_pool,
            tc.tile_pool(name="small", bufs=2) as small,
            tc.tile_pool(name="y_pool", bufs=3) as y_pool,
            tc.tile_pool(name="s_ps_pool", bufs=2, space="PSUM") as s_ps_pool,
            tc.tile_pool(name="acc_ps_pool", bufs=2, space="PSUM") as acc_ps_pool,
            tc.tile_pool(name="rb_ps_pool", bufs=2, space="PSUM") as rb_ps_pool,
        ):
            # Resident inputs: one big DMA each.
            qt_sb = persist.tile([P, HEADS_PER_CORE, N], F16)
            nc.sync.dma_start(out=qt_sb, in_=qt.rearrange("(h p) n -> p h n", p=P))
            kt_sb = persist.tile([P, HEADS_PER_CORE, S], F16)
            nc.sync.dma_start(out=kt_sb, in_=kt.rearrange("(h p) s -> p h s", p=P))
            v_sb = persist.tile([P, HEADS_PER_CORE, SC, HD], F16)
            nc.sync.dma_start(
                out=v_sb, in_=v.rearrange("(c p) (h d) -> p h c d", p=P, h=HEADS_PER_CORE)
            )
            wt_sb = persist.tile([P, HEADS_PER_CORE, DIM], F16)
            nc.sync.dma_start(out=wt_sb, in_=wt.rearrange("(h p) o -> p h o", p=P))

            ones_dram = nc.inline_tensor(np.ones((P, P), np.float16), name="ones_const")
            ones_mat = persist.tile([P, P], F16)
            nc.sync.dma_start(out=ones_mat, in_=ones_dram.ap())

            # X^T: normalized attention outputs, head-dim on partitions.
            xt_sb = persist.tile([P, HEADS_PER_CORE, N], F16)

            for h in range(HEADS_PER_CORE):
                for nq in range(NQ):
                    q_blk = qt_sb[:, h, nq * NB : (nq + 1) * NB]
                    o_ps = acc_ps_pool.tile([P, NB], F32, tag="acc")
                    a2 = a_pool.tile([P, 2, NB], F16, tag="a2")
                    for g in range(SC // 2):
                        s_ps = s_ps_pool.tile([P, 2, NB], F32, tag="s")
                        for j in range(2):
                            si = 2 * g + j
                            nc.tensor.matmul(
                                s_ps[:, j, :],
                                kt_sb[:, h, si * P : (si + 1) * P],
                                q_blk,
                                start=True,
                                stop=True,
                            )
                        e_t = e_pool.tile([P, 2, NB], F16, tag="e")
                        nc.scalar.activation(out=e_t, in_=s_ps, func=EXPF)
                        for j in range(2):
                            si = 2 * g + j
                            nc.tensor.matmul(
                                o_ps,
                                v_sb[:, h, si, :],
                                e_t[:, j, :],
                                start=(si == 0),
                                stop=(si == SC - 1),
                            )
                        with nc.allow_low_precision(
                            reason="fp16 rowsum partials; r has ~2e3 magnitude, fp16 keeps ~3e-4 rel"
                        ):
                            if g == 0:
                                nc.vector.tensor_copy(a2, e_t)
                            else:
                                nc.vector.tensor_add(a2, a2, e_t)
                    a1 = a_pool.tile([P, NB], F16, tag="a1")
                    with nc.allow_low_precision(reason="fp16 rowsum partials"):
                        nc.vector.tensor_add(a1, a2[:, 0, :], a2[:, 1, :])
                    # all-ones matmul: colsum over partitions, broadcast to all 128
                    rb_ps = rb_ps_pool.tile([P, NB], F32, tag="rb")
                    nc.tensor.matmul(rb_ps, ones_mat, a1, start=True, stop=True)
                    recip = small.tile([P, NB], F32, tag="recip")
                    nc.vector.reciprocal(recip, rb_ps)
                    with nc.allow_low_precision(reason="fp16 attention output grid"):
                        nc.vector.tensor_mul(
                            xt_sb[:, h, nq * NB : (nq + 1) * NB], o_ps, recip
                        )

            # Partial projection: Y = X @ W^T over this core's 256 dims.
            for nt in range(N // P):
                y_sb = y_pool.tile([P, DIM], F32, tag="y")
                for ot in range(2):
                    y_ps = acc_ps_pool.tile([P, NB], F32, tag="acc")
                    for h in range(HEADS_PER_CORE):
                        nc.tensor.matmul(
                            y_ps,
                            xt_sb[:, h, nt * P : (nt + 1) * P],
                            wt_sb[:, h, ot * NB : (ot + 1) * NB],
                            start=(h == 0),
                            stop=(h == HEADS_PER_CORE - 1),
                        )
                    nc.vector.tensor_copy(y_sb[:, ot * NB : (ot + 1) * NB], y_ps)
                nc.sync.dma_start(out=out[nt * P : (nt + 1) * P, :], in_=y_sb)

    nc.compile()
    return nc


def kernel(query, key, value, proj_w, proj_b):
    if "nc" not in _nc_cache:
        _nc_cache["nc"] = _build()
    nc = _nc_cache["nc"]

    scale = float(HD) ** -0.5
    wt_full = np.ascontiguousarray(proj_w.T.astype(np.float32))  # [in, out]
    in_maps = []
    for core in range(N_CORES):
        b, hg = divmod(core, N_CORES // B)
        sl = slice(hg * HG, (hg + 1) * HG)
        in_maps.append(
            {
                "qt": np.ascontiguousarray((query[b].T[sl] * scale), dtype=np.float16),
                "kt": np.ascontiguousarray(key[b].T[sl], dtype=np.float16),
                "v": np.ascontiguousarray(value[b][:, sl], dtype=np.float16),
                "wt": np.ascontiguousarray(wt_full[sl], dtype=np.float16),
            }
        )

    res = run_bass_kernel_spmd(nc, in_maps, list(range(N_CORES)))

    out = np.zeros((B, N, DIM), dtype=np.float32)
    for core in range(N_CORES):
        b = core // (N_CORES // B)
        out[b] += res.results[core]["out"]
    out += proj_b.astype(np.float32)
    return out
